# revision 23
# baseline (speedup 1.0000x reference)
"""AdaFocalLoss on 8 Trainium2 NeuronCores (Bass/Tile, SPMD).

Data-parallel over the batch axis, per the sharding hint: each core gets
8192 of the 65536 logit rows, the 15-entry gamma table is replicated, and
the per-core partial sums are combined on the host (the gather/unshard
step; the reduction over rows is order-independent).

Per-core kernel structure:
  - Rows are assigned to (slot, partition) SORTED BY TARGET on the host:
    slot s holds the 128 rows whose targets sit near the s-th quantile of
    the target distribution, so each slot's 128 target logits fall inside
    a static 64-column window and the gather is one windowed scan.
  - The shard streams as 64 solo 512KB DMAs over the Sync HWDGE ring,
    which alone saturates the ~400 B/ns per-core DMA ceiling (measured;
    all 16 SDMA engines serve one queue at ~25.6 B/ns each).  Slot 0
    rides FIRST so ScalarE's pipeline starts ~2.5us earlier than with
    the early slots on the slower-starting scalar ring; the scalar ring
    carries only the consts block and slots 2-3 for the ramp.  The DMA
    order is pinned with 1ns-increment sim-time lower bounds because the
    sim-guided tile scheduler otherwise front-loads whichever fresh-
    buffer DMAs it fancies (it once ran slot 63 first, starving slot 0
    for 6us).  A 20-buffer io pool gives ~25us of stream runway so the
    engine-progress-counter waits on late triggers never couple compute
    hiccups back into the stream.
  - ScalarE exps every element ([128,1000] fp16 out, 1.13us measured).
    Ten slots around the tail-part issue points plus the last two run as
    accum-slots: ACTIVATE accum_out row-sums are FREE on top of the exp
    (measured equal duration; the accumulator read costs 0.28us), and
    they relieve VectorE exactly where the tail-part chains land.
  - VectorE per V-slot: ONE fused scalar_tensor_tensor does the fp16
    halves-add AND accumulates the full row sum into the part's sumexp
    column (out=(h1*1)+h2, accum_out=sum; 0.68us - the broken
    tensor_tensor_reduce's job done with a working opcode); plus the
    windowed gather STT (iota64 == target_rel) * x with accum_out.
    End-stretch duty: ScalarE ~87%, VectorE ~76% of the 1.28us/slot
    arrival cadence - both keep up, no post-stream backlog.
  - Tail per row (5 unequal parts, each issued the moment its slots are
    done; only the last, 3 slots wide, is exposed past the stream):
    logpt = x_t - ln(sumexp), pt = exp(logpt) (the reference's exact
    formulation - no reciprocal), the telescoped gamma staircase
    sum_b dg_b*[pt >= b/15] with ds|dm fused into one [P,F,2,NB] mult +
    one reduce, then loss = -(1+eps-s*pt)^m * logpt via exp(m*ln(u));
    per-part row-sum partials land in one [128,5] tile shipped by a
    single DMA; the host sums and negates.

The gather windows are data-independent quantile bands (+-32 columns
~ 5.8 sigma of the sampling deviation for iid targets).  If an unusual
target distribution falls outside them, the host check catches it and
the kernel transparently rebuilds with full-width windows (slower but
always correct).
"""

import sys

for _p in ("/opt/trn_rl_repo",):
    if _p not in sys.path:
        sys.path.insert(0, _p)

import numpy as np

NUM_BINS = 15
EPS = 1e-20
N, C = 65536, 1000
NCORES = 8
NSHARD = N // NCORES  # 8192 rows per core
P = 128  # SBUF partitions
R = NSHARD // P  # 64 row-slots per partition
W = 64  # gather window width (columns) per row-slot
TAIL_BOUNDS = [0, 28, 44, 56, 61, 64]  # unequal tail parts; only the last
# (3 slots wide) is exposed past the stream, and its narrow chain runs in
# ~2.3us instead of the 3.5us an 8-wide part took
NPART = len(TAIL_BOUNDS) - 1
IO_BUFS = 20  # [P, 4000B] solo-slot buffers: 25us of stream runway, so
# the engine-progress-counter waits on late DMA triggers (buf recycle is
# expressed as "Activation_44 >= N") never couple tail-part hiccups back
# into the stream
EO_BUFS = 8

# Slots whose row-sum rides the ACTIVATE accumulator (free on ScalarE).
# They form AA chunk-pairs placed right AFTER each tail-part boundary, so
# the part's ~2us VectorE chain runs inside the AA islands' V-idle.  The
# end stretch (56-62) is all V-rowsum: with the fused STT row-sum
# (halves-add + accumulate in one ~0.76us op) VectorE runs ~76% duty
# there while ScalarE does plain Exp at ~87% - both keep up with the
# 1.28us/slot arrivals, which no A/V split of the old two-op row-sum
# could (measured: Exp(1000)=1.11us, accum read 0.28us, STT(500)=0.68us).
# Slots 62-63 are A-slots so no VectorE work gates the final tail chain.
ACT_SLOTS = frozenset({28, 29, 30, 44, 45, 46, 54, 55, 62, 63})

# DMA schedule: all SOLO 512KB transfers.  1MB two-slot chunks measured
# BIMODAL: most reps stream at only ~340 B/ns (vs ~400 solo) - the
# two-segments-per-partition descriptor pattern appears to hit an HBM
# interleave pathology.  Solo transfers were always tight at ~400+.
SYNC_RUNS = [(0, 1), (1, 1)] + [(s, 1) for s in range(4, 64)]
SCALAR_RUNS = [(2, 1), (3, 1)]


def _slot_lo(w):
    # static window starts: slot s is centered on the s-th target quantile
    return [min(max(int(C * (s + 0.5) / R) - w // 2, 0), C - w) for s in range(R)]


def _split_excess_waits(nc, mybir, max_waits=1):
    """This container's walrus supports only one sync-wait command per
    instruction; hoist extra waits onto preceding same-engine no-ops."""
    ctr = 0
    for f in nc.m.functions:
        for bb in f.blocks:
            new_insts = []
            changed = False
            for inst in bb.instructions:
                si = inst.sync_info
                if si is not None and si.on_wait and len(si.on_wait) > max_waits:
                    waits = list(si.on_wait)
                    excess, keep = waits[:-max_waits], waits[-max_waits:]
                    for i in range(0, len(excess), max_waits):
                        ctr += 1
                        new_insts.append(
                            mybir.InstNoOp(
                                name=f"I-waitsplit-{ctr}",
                                sync_info=mybir.SyncInfo(
                                    on_wait=list(excess[i : i + max_waits]),
                                    on_update=[],
                                ),
                                bass_nofuse=True,
                                engine=inst.engine,
                            )
                        )
                    si.on_wait = keep
                    changed = True
                new_insts.append(inst)
            if changed:
                bb.instructions[:] = new_insts


def _build(w):
    import concourse.bass as bass
    import concourse.tile as tile
    from concourse import mybir

    f32 = mybir.dt.float32
    f16 = mybir.dt.float16
    AF = mybir.ActivationFunctionType
    ALU = mybir.AluOpType
    NB = NUM_BINS
    slot_lo = _slot_lo(w)
    CW = w + R + 3 * NB  # consts layout: iota64 | tmap_rel | ds | dm | thr

    nc = bass.Bass()
    x = nc.declare_dram_parameter("x", [NSHARD, C], f32, isOutput=False)
    consts = nc.declare_dram_parameter("consts", [P, CW], f32, isOutput=False)
    out = nc.declare_dram_parameter("out", [P, NPART], f32, isOutput=True)

    # target-sorted rank-major layout: HBM row s*128 + p holds the row for
    # slot s, partition p, so any run of slots is one contiguous DMA.
    # Partition dim stays OUTERMOST on both sides of every dma_start.
    xs = x[:].rearrange("(s p) c -> s p c", p=P)

    def xrun(s, q):
        # [P, q, C] view of slots s..s+q-1 (s must be divisible by q)
        assert s % q == 0
        return x[:].rearrange("(u q p) c -> u p q c", q=q, p=P)[s // q]

    def slot_part(slot):
        h = 0
        while slot >= TAIL_BOUNDS[h + 1]:
            h += 1
        return h, slot - TAIL_BOUNDS[h]

    part_w = [TAIL_BOUNDS[h + 1] - TAIL_BOUNDS[h] for h in range(NPART)]

    with tile.TileContext(nc) as tc:
        with (
            tc.tile_pool(name="const", bufs=1) as cpool,
            tc.tile_pool(name="io", bufs=IO_BUFS) as iopool,
            tc.tile_pool(name="escr", bufs=EO_BUFS) as epool,
            tc.tile_pool(name="sscr", bufs=6) as spool,
            tc.tile_pool(name="acc", bufs=1) as apool,
            tc.tile_pool(name="tail", bufs=2) as tpool,
        ):
            # dummy 1-col activation first on the Scalar stream: the
            # compiler places the 1.28us ACT table load before the first
            # ACTIVATE in engine order, so this hoists it ahead of the
            # scalar ring's DMA triggers (input is a memset scratch tile)
            dz = cpool.tile([P, 1], f32, tag="dz", name="dz")
            nc.gpsimd.memset(dz[:], 0.0)
            ddum = cpool.tile([P, 1], f32, tag="ddum", name="ddum")
            nc.scalar.activation(ddum[:], dz[:], AF.Exp)

            # scalar ring: consts first (VectorE needs them early), then
            # slots 2-3 to help fill the ramp while the sync ring starts
            # slots 0-1.
            ct = cpool.tile([P, CW], f32, tag="consts")
            nc.scalar.dma_start(ct[:], consts[:])
            early = {}
            for s, q in SCALAR_RUNS:
                et = iopool.tile([P, q * C], f32, tag="xtile", name=f"xtile_e{s}")
                nc.scalar.dma_start(
                    et[:].rearrange("p (q c) -> p q c", q=q), xrun(s, q)
                )
                early[s] = et
            iota64 = ct[:, 0:w]
            tmap_t = ct[:, w : w + R]
            dsdm = ct[:, w + R : w + R + 2 * NB]  # ds then dm, adjacent
            thr = ct[:, w + R + 2 * NB : w + R + 3 * NB]

            # per-part accumulators so each tail part only depends on its
            # own stretch of the main loop
            sumexp = [
                apool.tile([P, part_w[h]], f32, tag=f"sumexp{h}", name=f"sumexp{h}")
                for h in range(NPART)
            ]
            xt = [
                apool.tile([P, part_w[h]], f32, tag=f"xt{h}", name=f"xt{h}")
                for h in range(NPART)
            ]
            # all four per-part row-sum columns land here; ONE DMA at the
            # end ships them
            rs_all = apool.tile([P, NPART], f32, tag="rs_all", name="rs_all")

            def tail_part(h):
                se, xh = sumexp[h], xt[h]
                F = part_w[h]
                # logpt = x_t - ln(sumexp);  pt = exp(logpt)  (identical to
                # the reference's formulation - no reciprocal needed)
                lse = tpool.tile([P, F], f32, tag="lse")
                nc.scalar.activation(lse[:], se[:], AF.Ln)
                logpt = tpool.tile([P, F], f32, tag="logpt")
                nc.vector.tensor_sub(logpt[:], xh[:], lse[:])
                pt = tpool.tile([P, F], f32, tag="pt")
                nc.scalar.activation(pt[:], logpt[:], AF.Exp)

                # s(pt), m(pt) via broadcast APs: ge[p,j,b] = pt[p,j]>=thr[p,b]
                ge = tpool.tile([P, F * NB], f32, tag="ge")
                ge3 = ge[:].rearrange("p (f b) -> p f b", b=NB)
                pt_b = (
                    pt[:]
                    .rearrange("p (f one) -> p f one", one=1)
                    .broadcast_to([P, F, NB])
                )
                thr_b = thr.rearrange("p (one b) -> p one b", one=1).broadcast_to(
                    [P, F, NB]
                )
                nc.vector.tensor_tensor(ge3, pt_b, thr_b, ALU.is_ge)
                # telescoped ds|dm staircase fused: one mult and one reduce
                # over a [P, F, 2, NB] view (ge broadcast over the pair axis)
                ge4 = (
                    ge[:]
                    .rearrange("p (f one b) -> p f one b", one=1, b=NB)
                    .broadcast_to([P, F, 2, NB])
                )
                dsdm_b = dsdm.rearrange(
                    "p (one two b) -> p one two b", one=1, two=2
                ).broadcast_to([P, F, 2, NB])
                prods2 = tpool.tile([P, F * 2 * NB], f32, tag="prods2")
                nc.vector.tensor_tensor(
                    prods2[:].rearrange("p (f two b) -> p f two b", two=2, b=NB),
                    ge4,
                    dsdm_b,
                    ALU.mult,
                )
                sm = tpool.tile([P, F * 2], f32, tag="sm")
                nc.vector.tensor_reduce(
                    sm[:], prods2[:].rearrange("p (ft b) -> p ft b", b=NB),
                    mybir.AxisListType.X, ALU.add,
                )
                sm3 = sm[:].rearrange("p (f two) -> p f two", two=2)
                s_acc = sm3[:, :, 0:1]
                m_acc = sm3[:, :, 1:2]
                pt3 = pt[:].rearrange("p (f one) -> p f one", one=1)

                # u = 1 + eps - s*pt ;  y = u^m = exp(m * ln(u)); the
                # "+1+eps" rides the Ln activation's bias immediate
                nspt = tpool.tile([P, F], f32, tag="nspt")
                nspt3 = nspt[:].rearrange("p (f one) -> p f one", one=1)
                nc.vector.scalar_tensor_tensor(
                    nspt3, s_acc, -1.0, pt3, ALU.mult, ALU.mult
                )
                v = tpool.tile([P, F], f32, tag="v")
                nc.scalar.activation(v[:], nspt[:], AF.Ln, bias=1.0 + EPS)
                w_ = tpool.tile([P, F], f32, tag="w")
                nc.vector.tensor_tensor(
                    w_[:].rearrange("p (f one) -> p f one", one=1),
                    v[:].rearrange("p (f one) -> p f one", one=1),
                    m_acc,
                    ALU.mult,
                )
                y = tpool.tile([P, F], f32, tag="y")
                nc.scalar.activation(y[:], w_[:], AF.Exp)

                # per-partition partial of sum_j y*logpt (negated on host),
                # multiply and row-sum fused via STT accum_out
                prod = tpool.tile([P, F], f32, tag="prod")
                nc.vector.scalar_tensor_tensor(
                    prod[:], y[:], 1.0, logpt[:], ALU.mult, ALU.mult,
                    accum_out=rs_all[:, h : h + 1],
                )

            def do_gather(slot, xtile, off):
                # rows are target-sorted, so this slot's targets all sit
                # inside a static window: the gather scans only it, against
                # window-relative indices (iota64 vs target - window_lo)
                h, col = slot_part(slot)
                lo = slot_lo[slot]
                so = spool.tile([P, w], f32, tag="so")
                nc.vector.scalar_tensor_tensor(
                    so[:],
                    iota64,
                    tmap_t[:, slot : slot + 1],
                    xtile[:, off + lo : off + lo + w],
                    ALU.is_equal,
                    ALU.mult,
                    accum_out=xt[h][:, col : col + 1],
                )

            def do_vrowsum(slot, eo, eoff):
                # V-slot row-sum, fused: ONE scalar_tensor_tensor does the
                # fp16 halves-add AND accumulates the full row sum into the
                # part's sumexp column (out = (h1*1)+h2, accum_out = sum)
                h, col = slot_part(slot)
                eh = epool.tile([P, C // 2], f16, tag="eh")
                nc.vector.scalar_tensor_tensor(
                    eh[:],
                    eo[:, eoff : eoff + C // 2],
                    1.0,
                    eo[:, eoff + C // 2 : eoff + C],
                    ALU.mult,
                    ALU.add,
                    accum_out=sumexp[h][:, col : col + 1],
                )

            def do_run(s0, q, xtile):
                # exp + row-sums for the q slots in this chunk, then gathers
                plain = [s0 + i for i in range(q) if (s0 + i) not in ACT_SLOTS]
                acts = [s0 + i for i in range(q) if (s0 + i) in ACT_SLOTS]
                eo = epool.tile([P, q * C], f16, tag="eo")
                if len(plain) == q and q == 2:
                    # one wide ACTIVATE covers both slots (measured cheaper)
                    nc.scalar.activation(eo[:], xtile[:, 0 : q * C], AF.Exp)
                else:
                    for s in plain:
                        o = (s - s0) * C
                        nc.scalar.activation(
                            eo[:, o : o + C], xtile[:, o : o + C], AF.Exp
                        )
                for s in acts:
                    o = (s - s0) * C
                    h, col = slot_part(s)
                    nc.scalar.activation(
                        eo[:, o : o + C], xtile[:, o : o + C], AF.Exp,
                        accum_out=sumexp[h][:, col : col + 1],
                    )
                for s in plain:
                    do_vrowsum(s, eo, (s - s0) * C)
                for s in range(s0, s0 + q):
                    do_gather(s, xtile, (s - s0) * C)

            done_parts = set()
            emitted = 0

            def maybe_tails(last_slot_done):
                for hh in range(NPART - 1):
                    if last_slot_done >= TAIL_BOUNDS[hh + 1] - 1 and hh not in done_parts:
                        done_parts.add(hh)
                        tail_part(hh)  # overlaps the rest of the stream

            # The sim-guided tile scheduler reorders same-engine ready
            # instructions (it front-loaded the final slots' fresh-buffer
            # DMAs, starving slot 0 for ~6us).  Pin the stream order with
            # 1ns-increment sim-time lower bounds per run.
            for i, (s, q) in enumerate(SYNC_RUNS):
                if s in early:
                    xtile = early[s]
                else:
                    with tc.tile_wait_until(i * 1e-6):
                        xtile = iopool.tile([P, q * C], f32, tag="xtile")
                        nc.sync.dma_start(
                            xtile[:].rearrange("p (q c) -> p q c", q=q),
                            xrun(s, q),
                        )
                do_run(s, q, xtile)
                # the early scalar-ring slots 2-3 logically follow slot 1
                if s == 1:
                    for es, eq in SCALAR_RUNS:
                        do_run(es, eq, early[es])
                    maybe_tails(3)
                else:
                    maybe_tails(s + q - 1)
            tail_part(NPART - 1)
            nc.sync.dma_start(out[:], rs_all[:])

    _split_excess_waits(nc, mybir, max_waits=1)
    return nc


_NC_CACHE = {}


def _get_nc(w):
    key = (w, tuple(SYNC_RUNS), tuple(sorted(ACT_SLOTS)), tuple(TAIL_BOUNDS), IO_BUFS)
    if key not in _NC_CACHE:
        _NC_CACHE[key] = _build(w)
    return _NC_CACHE[key]


def _make_in_maps(input, target, gammas, w):
    inp = np.ascontiguousarray(np.asarray(input, dtype=np.float32))
    tgt = np.asarray(target).astype(np.int64)
    gam = np.asarray(gammas, dtype=np.float32)
    assert inp.shape == (N, C) and tgt.shape == (N,) and gam.shape == (NUM_BINS,)

    slot_lo = np.asarray(_slot_lo(w), dtype=np.int64)

    # host-precomputed constant block: iota64 | tmap_rel | ds | dm | thr
    sgn = np.sign(gam)
    mag = np.abs(gam)
    ds = np.concatenate([sgn[:1], sgn[1:] - sgn[:-1]]).astype(np.float32)
    dm = np.concatenate([mag[:1], mag[1:] - mag[:-1]]).astype(np.float32)
    thr = (np.arange(NUM_BINS, dtype=np.float32) / np.float32(NUM_BINS)).astype(
        np.float32
    )
    iota_w = np.arange(w, dtype=np.float32)

    in_maps = []
    for i in range(NCORES):
        tshard = tgt[NSHARD * i : NSHARD * (i + 1)]
        # sort rows by target; rank r -> slot r//P, partition r%P, so each
        # slot's 128 targets fall inside its static gather window
        order = np.argsort(tshard, kind="stable")
        tsorted = tshard[order]
        by_slot = tsorted.reshape(R, P)  # [slot, partition]
        lo = slot_lo[:, None]
        if not np.all((by_slot >= lo) & (by_slot <= lo + (w - 1))):
            return None  # caller falls back to full-width windows
        shard = np.ascontiguousarray(inp[NSHARD * i : NSHARD * (i + 1)][order])
        tmap_rel = (by_slot - lo).T.astype(np.float32)  # [P, R], in [0, w)
        row = np.concatenate([iota_w, np.zeros(R, np.float32), ds, dm, thr])
        consts = np.broadcast_to(row, (P, row.size)).copy()
        consts[:, w : w + R] = tmap_rel
        in_maps.append({"x": shard, "consts": np.ascontiguousarray(consts)})
    return in_maps


def kernel(input, target, gammas, _trace=False, _tmpdir=None):
    from concourse.bass_utils import run_bass_kernel_spmd

    in_maps = _make_in_maps(input, target, gammas, W)
    w = W
    if in_maps is None:
        # pathological target distribution: use full-width gather windows
        w = C
        in_maps = _make_in_maps(input, target, gammas, w)
        assert in_maps is not None  # w == C always satisfies the window check

    res = run_bass_kernel_spmd(
        _get_nc(w),
        in_maps,
        core_ids=list(range(NCORES)),
        trace=_trace,
        tmpdir=_tmpdir,
    )
    partials = [float(np.sum(res.results[i]["out"])) for i in range(NCORES)]
    total = -np.float32(np.sum(np.asarray(partials, dtype=np.float32)))
    if _trace:
        kernel._last_result = res
    return np.array(total, dtype=np.float32)


# revision 25
# speedup vs baseline: 1.0977x; 1.0977x over previous
"""AdaFocalLoss on 8 Trainium2 NeuronCores (Bass/Tile, SPMD).

Data-parallel over the batch axis, per the sharding hint: each core gets
8192 of the 65536 logit rows, the 15-entry gamma table is replicated, and
the per-core partial sums are combined on the host (the gather/unshard
step; the reduction over rows is order-independent).

Per-core kernel structure:
  - Rows are assigned to (slot, partition) SORTED BY TARGET on the host:
    slot s holds the 128 rows whose targets sit near the s-th quantile of
    the target distribution, so each slot's 128 target logits fall inside
    a static 64-column window and the gather is one windowed scan.
  - The shard streams as 64 solo 512KB DMAs over the Sync HWDGE ring,
    which alone saturates the ~400 B/ns per-core DMA ceiling (measured;
    all 16 SDMA engines serve one queue at ~25.6 B/ns each).  Slot 0
    rides FIRST so ScalarE's pipeline starts ~2.5us earlier than with
    the early slots on the slower-starting scalar ring; the scalar ring
    carries only the consts block and slots 2-3 for the ramp.  The DMA
    order is pinned with 1ns-increment sim-time lower bounds because the
    sim-guided tile scheduler otherwise front-loads whichever fresh-
    buffer DMAs it fancies (it once ran slot 63 first, starving slot 0
    for 6us).  A 20-buffer io pool gives ~25us of stream runway so the
    engine-progress-counter waits on late triggers never couple compute
    hiccups back into the stream.
  - ScalarE exps every element ([128,1000] fp16 out, 1.13us measured).
    Ten slots around the tail-part issue points plus the last two run as
    accum-slots: ACTIVATE accum_out row-sums are FREE on top of the exp
    (measured equal duration; the accumulator read costs 0.28us), and
    they relieve VectorE exactly where the tail-part chains land.
  - VectorE per V-slot: ONE fused scalar_tensor_tensor does the fp16
    halves-add AND accumulates the full row sum into the part's sumexp
    column (out=(h1*1)+h2, accum_out=sum; 0.68us - the broken
    tensor_tensor_reduce's job done with a working opcode); plus the
    windowed gather STT (iota64 == target_rel) * x with accum_out.
    End-stretch duty: ScalarE ~87%, VectorE ~76% of the 1.28us/slot
    arrival cadence - both keep up, no post-stream backlog.
  - Tail per row (5 unequal parts, each issued the moment its slots are
    done; only the last, 3 slots wide, is exposed past the stream):
    logpt = x_t - ln(sumexp), pt = exp(logpt) (the reference's exact
    formulation - no reciprocal), the telescoped gamma staircase
    sum_b dg_b*[pt >= b/15] with ds|dm fused into one [P,F,2,NB] mult +
    one reduce, then loss = -(1+eps-s*pt)^m * logpt via exp(m*ln(u));
    per-part row-sum partials land in one [128,5] tile shipped by a
    single DMA; the host sums and negates.

The gather windows are data-independent quantile bands (+-32 columns
~ 5.8 sigma of the sampling deviation for iid targets).  If an unusual
target distribution falls outside them, the host check catches it and
the kernel transparently rebuilds with full-width windows (slower but
always correct).
"""

import sys

for _p in ("/opt/trn_rl_repo",):
    if _p not in sys.path:
        sys.path.insert(0, _p)

import numpy as np

NUM_BINS = 15
EPS = 1e-20
N, C = 65536, 1000
NCORES = 8
NSHARD = N // NCORES  # 8192 rows per core
P = 128  # SBUF partitions
R = NSHARD // P  # 64 row-slots per partition
W = 64  # gather window width (columns) per row-slot
TAIL_BOUNDS = [0, 28, 44, 56, 61, 64]  # unequal tail parts; only the last
# (3 slots wide) is exposed past the stream, and its narrow chain runs in
# ~2.3us instead of the 3.5us an 8-wide part took
NPART = len(TAIL_BOUNDS) - 1
IO_BUFS = 20  # [P, 4000B] solo-slot buffers: 25us of stream runway, so
# the engine-progress-counter waits on late DMA triggers (buf recycle is
# expressed as "Activation_44 >= N") never couple tail-part hiccups back
# into the stream
EO_BUFS = 8

# Slots whose row-sum rides the ACTIVATE accumulator (free on ScalarE).
# They form AA chunk-pairs placed right AFTER each tail-part boundary, so
# the part's ~2us VectorE chain runs inside the AA islands' V-idle.  The
# end stretch (56-62) is all V-rowsum: with the fused STT row-sum
# (halves-add + accumulate in one ~0.76us op) VectorE runs ~76% duty
# there while ScalarE does plain Exp at ~87% - both keep up with the
# 1.28us/slot arrivals, which no A/V split of the old two-op row-sum
# could (measured: Exp(1000)=1.11us, accum read 0.28us, STT(500)=0.68us).
# Slots 62-63 are A-slots so no VectorE work gates the final tail chain.
# Slot 63 streams as two HALF-slot DMAs (order-pinned, so the scheduler
# hoist that broke this earlier cannot recur) and runs two 500-col accum
# ACTIVATEs: only ~0.63us of Exp is exposed past the last byte, not 1.11.
ACT_SLOTS = frozenset({28, 29, 30, 44, 45, 46, 54, 55, 62})

# DMA schedule: all SOLO 512KB transfers.  1MB two-slot chunks measured
# BIMODAL: most reps stream at only ~340 B/ns (vs ~400 solo) - the
# two-segments-per-partition descriptor pattern appears to hit an HBM
# interleave pathology.  Solo transfers were always tight at ~400+.
SYNC_RUNS = [(0, 1), (1, 1)] + [(s, 1) for s in range(4, 63)]
SCALAR_RUNS = [(2, 1), (3, 1)]


def _slot_lo(w):
    # static window starts: slot s is centered on the s-th target quantile
    return [min(max(int(C * (s + 0.5) / R) - w // 2, 0), C - w) for s in range(R)]


def _split_excess_waits(nc, mybir, max_waits=1):
    """This container's walrus supports only one sync-wait command per
    instruction; hoist extra waits onto preceding same-engine no-ops."""
    ctr = 0
    for f in nc.m.functions:
        for bb in f.blocks:
            new_insts = []
            changed = False
            for inst in bb.instructions:
                si = inst.sync_info
                if si is not None and si.on_wait and len(si.on_wait) > max_waits:
                    waits = list(si.on_wait)
                    excess, keep = waits[:-max_waits], waits[-max_waits:]
                    for i in range(0, len(excess), max_waits):
                        ctr += 1
                        new_insts.append(
                            mybir.InstNoOp(
                                name=f"I-waitsplit-{ctr}",
                                sync_info=mybir.SyncInfo(
                                    on_wait=list(excess[i : i + max_waits]),
                                    on_update=[],
                                ),
                                bass_nofuse=True,
                                engine=inst.engine,
                            )
                        )
                    si.on_wait = keep
                    changed = True
                new_insts.append(inst)
            if changed:
                bb.instructions[:] = new_insts


def _build(w):
    import concourse.bass as bass
    import concourse.tile as tile
    from concourse import mybir

    f32 = mybir.dt.float32
    f16 = mybir.dt.float16
    AF = mybir.ActivationFunctionType
    ALU = mybir.AluOpType
    NB = NUM_BINS
    slot_lo = _slot_lo(w)
    CW = w + R + 3 * NB  # consts layout: iota64 | tmap_rel | ds | dm | thr

    nc = bass.Bass()
    x = nc.declare_dram_parameter("x", [NSHARD, C], f32, isOutput=False)
    consts = nc.declare_dram_parameter("consts", [P, CW], f32, isOutput=False)
    out = nc.declare_dram_parameter("out", [P, NPART], f32, isOutput=True)

    # target-sorted rank-major layout: HBM row s*128 + p holds the row for
    # slot s, partition p, so any run of slots is one contiguous DMA.
    # Partition dim stays OUTERMOST on both sides of every dma_start.
    xs = x[:].rearrange("(s p) c -> s p c", p=P)

    def xrun(s, q):
        # [P, q, C] view of slots s..s+q-1 (s must be divisible by q)
        assert s % q == 0
        return x[:].rearrange("(u q p) c -> u p q c", q=q, p=P)[s // q]

    def slot_part(slot):
        h = 0
        while slot >= TAIL_BOUNDS[h + 1]:
            h += 1
        return h, slot - TAIL_BOUNDS[h]

    part_w = [TAIL_BOUNDS[h + 1] - TAIL_BOUNDS[h] for h in range(NPART)]

    with tile.TileContext(nc) as tc:
        with (
            tc.tile_pool(name="const", bufs=1) as cpool,
            tc.tile_pool(name="io", bufs=IO_BUFS) as iopool,
            tc.tile_pool(name="escr", bufs=EO_BUFS) as epool,
            tc.tile_pool(name="sscr", bufs=6) as spool,
            tc.tile_pool(name="acc", bufs=1) as apool,
            tc.tile_pool(name="tail", bufs=2) as tpool,
        ):
            # dummy 1-col activation first on the Scalar stream: the
            # compiler places the 1.28us ACT table load before the first
            # ACTIVATE in engine order, so this hoists it ahead of the
            # scalar ring's DMA triggers (input is a memset scratch tile)
            dz = cpool.tile([P, 1], f32, tag="dz", name="dz")
            nc.gpsimd.memset(dz[:], 0.0)
            ddum = cpool.tile([P, 1], f32, tag="ddum", name="ddum")
            nc.scalar.activation(ddum[:], dz[:], AF.Exp)

            # scalar ring: consts first (VectorE needs them early), then
            # slots 2-3 to help fill the ramp while the sync ring starts
            # slots 0-1.
            ct = cpool.tile([P, CW], f32, tag="consts")
            nc.scalar.dma_start(ct[:], consts[:])
            early = {}
            for s, q in SCALAR_RUNS:
                et = iopool.tile([P, q * C], f32, tag="xtile", name=f"xtile_e{s}")
                nc.scalar.dma_start(
                    et[:].rearrange("p (q c) -> p q c", q=q), xrun(s, q)
                )
                early[s] = et
            iota64 = ct[:, 0:w]
            tmap_t = ct[:, w : w + R]
            dsdm = ct[:, w + R : w + R + 2 * NB]  # ds then dm, adjacent
            thr = ct[:, w + R + 2 * NB : w + R + 3 * NB]

            # per-part accumulators so each tail part only depends on its
            # own stretch of the main loop
            sumexp = [
                apool.tile([P, part_w[h]], f32, tag=f"sumexp{h}", name=f"sumexp{h}")
                for h in range(NPART)
            ]
            xt = [
                apool.tile([P, part_w[h]], f32, tag=f"xt{h}", name=f"xt{h}")
                for h in range(NPART)
            ]
            # all four per-part row-sum columns land here; ONE DMA at the
            # end ships them
            rs_all = apool.tile([P, NPART], f32, tag="rs_all", name="rs_all")

            def tail_part(h):
                se, xh = sumexp[h], xt[h]
                F = part_w[h]
                # logpt = x_t - ln(sumexp);  pt = exp(logpt)  (identical to
                # the reference's formulation - no reciprocal needed)
                lse = tpool.tile([P, F], f32, tag="lse")
                nc.scalar.activation(lse[:], se[:], AF.Ln)
                logpt = tpool.tile([P, F], f32, tag="logpt")
                nc.vector.tensor_sub(logpt[:], xh[:], lse[:])
                pt = tpool.tile([P, F], f32, tag="pt")
                nc.scalar.activation(pt[:], logpt[:], AF.Exp)

                # s(pt), m(pt) via broadcast APs: ge[p,j,b] = pt[p,j]>=thr[p,b]
                ge = tpool.tile([P, F * NB], f32, tag="ge")
                ge3 = ge[:].rearrange("p (f b) -> p f b", b=NB)
                pt_b = (
                    pt[:]
                    .rearrange("p (f one) -> p f one", one=1)
                    .broadcast_to([P, F, NB])
                )
                thr_b = thr.rearrange("p (one b) -> p one b", one=1).broadcast_to(
                    [P, F, NB]
                )
                nc.vector.tensor_tensor(ge3, pt_b, thr_b, ALU.is_ge)
                # telescoped ds|dm staircase fused: one mult and one reduce
                # over a [P, F, 2, NB] view (ge broadcast over the pair axis)
                ge4 = (
                    ge[:]
                    .rearrange("p (f one b) -> p f one b", one=1, b=NB)
                    .broadcast_to([P, F, 2, NB])
                )
                dsdm_b = dsdm.rearrange(
                    "p (one two b) -> p one two b", one=1, two=2
                ).broadcast_to([P, F, 2, NB])
                prods2 = tpool.tile([P, F * 2 * NB], f32, tag="prods2")
                nc.vector.tensor_tensor(
                    prods2[:].rearrange("p (f two b) -> p f two b", two=2, b=NB),
                    ge4,
                    dsdm_b,
                    ALU.mult,
                )
                sm = tpool.tile([P, F * 2], f32, tag="sm")
                nc.vector.tensor_reduce(
                    sm[:], prods2[:].rearrange("p (ft b) -> p ft b", b=NB),
                    mybir.AxisListType.X, ALU.add,
                )
                sm3 = sm[:].rearrange("p (f two) -> p f two", two=2)
                s_acc = sm3[:, :, 0:1]
                m_acc = sm3[:, :, 1:2]
                pt3 = pt[:].rearrange("p (f one) -> p f one", one=1)

                # u = 1 + eps - s*pt ;  y = u^m = exp(m * ln(u)); the
                # "+1+eps" rides the Ln activation's bias immediate
                nspt = tpool.tile([P, F], f32, tag="nspt")
                nspt3 = nspt[:].rearrange("p (f one) -> p f one", one=1)
                nc.vector.scalar_tensor_tensor(
                    nspt3, s_acc, -1.0, pt3, ALU.mult, ALU.mult
                )
                v = tpool.tile([P, F], f32, tag="v")
                nc.scalar.activation(v[:], nspt[:], AF.Ln, bias=1.0 + EPS)
                w_ = tpool.tile([P, F], f32, tag="w")
                nc.vector.tensor_tensor(
                    w_[:].rearrange("p (f one) -> p f one", one=1),
                    v[:].rearrange("p (f one) -> p f one", one=1),
                    m_acc,
                    ALU.mult,
                )
                y = tpool.tile([P, F], f32, tag="y")
                nc.scalar.activation(y[:], w_[:], AF.Exp)

                # per-partition partial of sum_j y*logpt (negated on host),
                # multiply and row-sum fused via STT accum_out
                prod = tpool.tile([P, F], f32, tag="prod")
                nc.vector.scalar_tensor_tensor(
                    prod[:], y[:], 1.0, logpt[:], ALU.mult, ALU.mult,
                    accum_out=rs_all[:, h : h + 1],
                )

            def do_gather(slot, xtile, off):
                # rows are target-sorted, so this slot's targets all sit
                # inside a static window: the gather scans only it, against
                # window-relative indices (iota64 vs target - window_lo)
                h, col = slot_part(slot)
                lo = slot_lo[slot]
                so = spool.tile([P, w], f32, tag="so")
                nc.vector.scalar_tensor_tensor(
                    so[:],
                    iota64,
                    tmap_t[:, slot : slot + 1],
                    xtile[:, off + lo : off + lo + w],
                    ALU.is_equal,
                    ALU.mult,
                    accum_out=xt[h][:, col : col + 1],
                )

            def do_vrowsum(slot, eo, eoff):
                # V-slot row-sum, fused: ONE scalar_tensor_tensor does the
                # fp16 halves-add AND accumulates the full row sum into the
                # part's sumexp column (out = (h1*1)+h2, accum_out = sum)
                h, col = slot_part(slot)
                eh = epool.tile([P, C // 2], f16, tag="eh")
                nc.vector.scalar_tensor_tensor(
                    eh[:],
                    eo[:, eoff : eoff + C // 2],
                    1.0,
                    eo[:, eoff + C // 2 : eoff + C],
                    ALU.mult,
                    ALU.add,
                    accum_out=sumexp[h][:, col : col + 1],
                )

            def do_run(s0, q, xtile):
                # exp + row-sums for the q slots in this chunk, then gathers
                plain = [s0 + i for i in range(q) if (s0 + i) not in ACT_SLOTS]
                acts = [s0 + i for i in range(q) if (s0 + i) in ACT_SLOTS]
                eo = epool.tile([P, q * C], f16, tag="eo")
                if len(plain) == q and q == 2:
                    # one wide ACTIVATE covers both slots (measured cheaper)
                    nc.scalar.activation(eo[:], xtile[:, 0 : q * C], AF.Exp)
                else:
                    for s in plain:
                        o = (s - s0) * C
                        nc.scalar.activation(
                            eo[:, o : o + C], xtile[:, o : o + C], AF.Exp
                        )
                for s in acts:
                    o = (s - s0) * C
                    h, col = slot_part(s)
                    nc.scalar.activation(
                        eo[:, o : o + C], xtile[:, o : o + C], AF.Exp,
                        accum_out=sumexp[h][:, col : col + 1],
                    )
                for s in plain:
                    do_vrowsum(s, eo, (s - s0) * C)
                for s in range(s0, s0 + q):
                    do_gather(s, xtile, (s - s0) * C)

            done_parts = set()
            emitted = 0

            def maybe_tails(last_slot_done):
                for hh in range(NPART - 1):
                    if last_slot_done >= TAIL_BOUNDS[hh + 1] - 1 and hh not in done_parts:
                        done_parts.add(hh)
                        tail_part(hh)  # overlaps the rest of the stream

            # The sim-guided tile scheduler reorders same-engine ready
            # instructions (it front-loaded the final slots' fresh-buffer
            # DMAs, starving slot 0 for ~6us).  Pin the stream order with
            # 1ns-increment sim-time lower bounds per run.
            for i, (s, q) in enumerate(SYNC_RUNS):
                if s in early:
                    xtile = early[s]
                else:
                    with tc.tile_wait_until(i * 1e-6):
                        xtile = iopool.tile([P, q * C], f32, tag="xtile")
                        nc.sync.dma_start(
                            xtile[:].rearrange("p (q c) -> p q c", q=q),
                            xrun(s, q),
                        )
                do_run(s, q, xtile)
                # the early scalar-ring slots 2-3 logically follow slot 1
                if s == 1:
                    for es, eq in SCALAR_RUNS:
                        do_run(es, eq, early[es])
                    maybe_tails(3)
                else:
                    maybe_tails(s + q - 1)

            # slot 63: two order-pinned half-slot DMAs and two 500-col
            # accum ACTIVATEs (halved Exp exposure after the last byte); a
            # tiny V-add merges the accumulator halves.  The w=C fallback
            # keeps the plain solo form (its gather window straddles).
            h63, col63 = slot_part(63)
            if w != C:
                s63 = apool.tile([P, 2], f32, tag="s63", name="s63")
                with tc.tile_wait_until(len(SYNC_RUNS) * 1e-6):
                    x63a = iopool.tile([P, C // 2], f32, tag="xtile", name="x63a")
                    nc.sync.dma_start(x63a[:], xs[63, :, 0 : C // 2])
                with tc.tile_wait_until((len(SYNC_RUNS) + 1) * 1e-6):
                    x63b = iopool.tile([P, C // 2], f32, tag="xtile", name="x63b")
                    nc.sync.dma_start(x63b[:], xs[63, :, C // 2 : C])
                eo63 = epool.tile([P, C], f16, tag="eo", name="eo63")
                nc.scalar.activation(
                    eo63[:, 0 : C // 2], x63a[:], AF.Exp, accum_out=s63[:, 0:1]
                )
                nc.scalar.activation(
                    eo63[:, C // 2 : C], x63b[:], AF.Exp, accum_out=s63[:, 1:2]
                )
                nc.vector.tensor_add(
                    sumexp[h63][:, col63 : col63 + 1], s63[:, 0:1], s63[:, 1:2]
                )
                lo63 = slot_lo[63]
                so63 = spool.tile([P, w], f32, tag="so", name="so63")
                assert lo63 >= C // 2 or lo63 + w <= C // 2  # window in one half
                g63 = (
                    x63b[:, lo63 - C // 2 : lo63 - C // 2 + w]
                    if lo63 >= C // 2
                    else x63a[:, lo63 : lo63 + w]
                )
                nc.vector.scalar_tensor_tensor(
                    so63[:], iota64, tmap_t[:, 63:64], g63,
                    ALU.is_equal, ALU.mult,
                    accum_out=xt[h63][:, col63 : col63 + 1],
                )
            else:
                with tc.tile_wait_until(len(SYNC_RUNS) * 1e-6):
                    x63 = iopool.tile([P, C], f32, tag="xtile", name="x63")
                    nc.sync.dma_start(x63[:], xs[63, :, :])
                eo63 = epool.tile([P, C], f16, tag="eo", name="eo63")
                nc.scalar.activation(
                    eo63[:], x63[:], AF.Exp,
                    accum_out=sumexp[h63][:, col63 : col63 + 1],
                )
                so63 = spool.tile([P, w], f32, tag="so", name="so63")
                nc.vector.scalar_tensor_tensor(
                    so63[:], iota64, tmap_t[:, 63:64], x63[:, 0:w],
                    ALU.is_equal, ALU.mult,
                    accum_out=xt[h63][:, col63 : col63 + 1],
                )
            tail_part(NPART - 1)
            nc.sync.dma_start(out[:], rs_all[:])

    _split_excess_waits(nc, mybir, max_waits=1)
    return nc


_NC_CACHE = {}


def _get_nc(w):
    key = (w, tuple(SYNC_RUNS), tuple(sorted(ACT_SLOTS)), tuple(TAIL_BOUNDS), IO_BUFS)
    if key not in _NC_CACHE:
        _NC_CACHE[key] = _build(w)
    return _NC_CACHE[key]


def _make_in_maps(input, target, gammas, w):
    inp = np.ascontiguousarray(np.asarray(input, dtype=np.float32))
    tgt = np.asarray(target).astype(np.int64)
    gam = np.asarray(gammas, dtype=np.float32)
    assert inp.shape == (N, C) and tgt.shape == (N,) and gam.shape == (NUM_BINS,)

    slot_lo = np.asarray(_slot_lo(w), dtype=np.int64)

    # host-precomputed constant block: iota64 | tmap_rel | ds | dm | thr
    sgn = np.sign(gam)
    mag = np.abs(gam)
    ds = np.concatenate([sgn[:1], sgn[1:] - sgn[:-1]]).astype(np.float32)
    dm = np.concatenate([mag[:1], mag[1:] - mag[:-1]]).astype(np.float32)
    thr = (np.arange(NUM_BINS, dtype=np.float32) / np.float32(NUM_BINS)).astype(
        np.float32
    )
    iota_w = np.arange(w, dtype=np.float32)

    in_maps = []
    for i in range(NCORES):
        tshard = tgt[NSHARD * i : NSHARD * (i + 1)]
        # sort rows by target; rank r -> slot r//P, partition r%P, so each
        # slot's 128 targets fall inside its static gather window
        order = np.argsort(tshard, kind="stable")
        tsorted = tshard[order]
        by_slot = tsorted.reshape(R, P)  # [slot, partition]
        lo = slot_lo[:, None]
        if not np.all((by_slot >= lo) & (by_slot <= lo + (w - 1))):
            return None  # caller falls back to full-width windows
        shard = np.ascontiguousarray(inp[NSHARD * i : NSHARD * (i + 1)][order])
        tmap_rel = (by_slot - lo).T.astype(np.float32)  # [P, R], in [0, w)
        row = np.concatenate([iota_w, np.zeros(R, np.float32), ds, dm, thr])
        consts = np.broadcast_to(row, (P, row.size)).copy()
        consts[:, w : w + R] = tmap_rel
        in_maps.append({"x": shard, "consts": np.ascontiguousarray(consts)})
    return in_maps


def kernel(input, target, gammas, _trace=False, _tmpdir=None):
    from concourse.bass_utils import run_bass_kernel_spmd

    in_maps = _make_in_maps(input, target, gammas, W)
    w = W
    if in_maps is None:
        # pathological target distribution: use full-width gather windows
        w = C
        in_maps = _make_in_maps(input, target, gammas, w)
        assert in_maps is not None  # w == C always satisfies the window check

    res = run_bass_kernel_spmd(
        _get_nc(w),
        in_maps,
        core_ids=list(range(NCORES)),
        trace=_trace,
        tmpdir=_tmpdir,
    )
    partials = [float(np.sum(res.results[i]["out"])) for i in range(NCORES)]
    total = -np.float32(np.sum(np.asarray(partials, dtype=np.float32)))
    if _trace:
        kernel._last_result = res
    return np.array(total, dtype=np.float32)
